# revision 10
# baseline (speedup 1.0000x reference)
"""Trainium2 Bass/Tile kernel for the Calibrated Spectral Mixer.

Strategy (8 NeuronCores, pure data-parallel over batch, 4 samples/core):
  - The two 3x3 SAME convs are fused into one 9-tap shifted-matmul over a
    zero-padded (103, 33) image kept channel-major (cin on partitions) in
    SBUF as bf16. The conv is evaluated at ALL padded-grid centers in flat
    contiguous 128-wide tiles, so every matmul weight AP is 1-D (a walrus
    requirement); the ~6% garbage centers (pad rows/cols) are neutralized
    by zeroing the spectral basis there and skipping them in the output
    DMA.
  - x_mid is never materialized: gate_w, the per-head temperature and the
    gate bias are folded into an effective conv weight on the host, so the
    conv directly emits softmax logits.
  - Spatial stays on PSUM/SBUF partitions throughout, so softmax reduces
    along the free axis and the spectral einsums (contraction over n) need
    no transposes; only the final mix needs eig^T (PE transposes).
  - out_spec @ out_w is pre-contracted per head into M_h = out_spec_h @
    out_w_h^T on the PE, so the final stage is 4 K=128 matmuls per tile.
"""

import re
import sys
import types

import numpy as np

import bass_rust
import concourse.bass as bass
import concourse.bacc as bacc
import concourse.tile as tile
from concourse import mybir
from concourse.masks import make_identity
from concourse.vector_clock import ScopedClock

# ---------------------------------------------------------------------------
# Environment patches (this container's walrus build + axon client quirks).
# ---------------------------------------------------------------------------


def _patched_drain_and_barrier(self, tick_clock, wait_clock):
    # This walrus build rejects instructions with more than one sync wait;
    # the Tile kernel-tail drain waits on every active logical proc. Split
    # it into one drain per proc.
    gc = tick_clock.global_clock
    vals = [int(v) for v in re.search(r"\[([^\]]*)\]", str(gc)).group(1).split(",")]
    nz = [i for i, v in enumerate(vals) if v > 0]
    for j in nz:
        vec = [0] * len(vals)
        vec[j] = vals[j]
        drain_inst = self.nc.sync.drain()
        wait_clock.add_sem_waits(
            drain_inst.ins, ScopedClock({None: bass_rust.VectorClock(vec)})
        )
    if not nz:
        self.nc.sync.drain()
    self.nc.all_engine_barrier()
    assert self.sems is not None
    popped = self.nc._tile_sem_poison_stack.pop()
    assert popped is self._sem_poison
    self.nc.clear_and_free_semaphores(list(self.sems.allocated().values()))
    self.nc.all_engine_barrier()


# Installed only as a fallback; Bacc.compile()'s generate_event_semaphores is
# the official wait-splitting pass, so the stock drain is fine under Bacc.
# tile.TileContext._drain_and_barrier = _patched_drain_and_barrier


def _install_ntff_hook():
    # antenv.axon_hooks is missing on this image; recreate the registry and
    # register the ctypes NTFF hook so trace=True works for profiling.
    import antenv

    if "antenv.axon_hooks" in sys.modules:
        return
    mod = types.ModuleType("antenv.axon_hooks")
    mod._hook = None
    mod.set_axon_ntff_profile_hook = lambda h: setattr(mod, "_hook", h)
    mod.get_axon_ntff_profile_hook = lambda: mod._hook
    sys.modules["antenv.axon_hooks"] = mod
    antenv.axon_hooks = mod
    try:
        from trn_agent_boot.trn_boot import _ntff_profile_via_ctypes

        mod.set_axon_ntff_profile_hook(
            _ntff_profile_via_ctypes("/opt/axon/libaxon_pjrt.so")
        )
    except Exception:
        pass

    import concourse.bass_utils as bass_utils

    bass_utils.upload_artifacts = lambda tmpdir: f"file://{tmpdir}"


_install_ntff_hook()

from concourse.bass_utils import run_bass_kernel_spmd  # noqa: E402

# ---------------------------------------------------------------------------
# Problem constants.
# ---------------------------------------------------------------------------

HH, WW = 101, 31
N = HH * WW  # 3131
C = 256
HEADS, DH, FREQ = 8, 64, 64
INNER = HEADS * DH  # 512
EPS = 1e-5
B = 32
NCORES = 8
BPC = B // NCORES  # 4 samples per core
PH, PW = HH + 2, WW + 2  # padded image (103, 33)
NP = PH * PW  # 3399 padded centers
GUARD = 34  # flat guard cells on each side (max |tap offset| = 33 + 1)
XLEN = NP + 2 * GUARD  # padded+guarded flat length

# x-input row-tiles (valid image rows): 25 x 4 rows (124 pos) + 1 x 1 row.
IN_TILES = [(4 * t, 4, 124) for t in range(25)] + [(100, 1, 31)]

# conv/output tiles over the flat padded grid.
CONV_TILES = [(128 * t, min(128, NP - 128 * t)) for t in range((NP + 127) // 128)]
NT = len(CONV_TILES)  # 27

# Valid-output segments per conv tile: (offset within tile, y row, length).
OUT_SEGS = []
for p0, sz in CONV_TILES:
    segs = []
    for pi in range(1, PH - 1):
        a = pi * PW + 1  # first valid flat pos of this image row
        b = a + WW  # one past last
        lo, hi = max(a, p0), min(b, p0 + sz)
        if lo < hi:
            segs.append((lo - p0, (pi - 1) * WW + (lo - a), hi - lo))
    OUT_SEGS.append(segs)

F32 = mybir.dt.float32
BF16 = mybir.dt.bfloat16

TAPS = [(di, dj) for di in range(3) for dj in range(3)]


def _build_nc(use_brow=True, use_obrow=True):
    nc = bacc.Bacc()

    x_d = nc.declare_dram_parameter("x", [BPC, N, C], F32, isOutput=False)
    wall_d = nc.declare_dram_parameter("wall", [2, 128, 9, 1024], BF16, isOutput=False)
    inver_d = nc.declare_dram_parameter("inver_p", [128, NT, 1, FREQ], F32, isOutput=False)
    mlp_d = nc.declare_dram_parameter("mlp_b", [DH, DH], BF16, isOutput=False)
    gamma_d = nc.declare_dram_parameter("gamma_t", [DH, FREQ], F32, isOutput=False)
    beta_d = nc.declare_dram_parameter("beta_t", [DH, FREQ], F32, isOutput=False)
    wo_d = nc.declare_dram_parameter("wo_t", [DH, HEADS, C], BF16, isOutput=False)
    brow_d = nc.declare_dram_parameter("bias_row", [1, 1024], BF16, isOutput=False)
    obrow_d = nc.declare_dram_parameter("outb_row", [1, C], BF16, isOutput=False)
    y_d = nc.declare_dram_parameter("y", [BPC, N, C], F32, isOutput=True)

    with tile.TileContext(nc) as tc:
        with (
            tc.tile_pool(name="singles", bufs=1) as singles,
            tc.tile_pool(name="xin", bufs=3) as xin_pool,
            tc.tile_pool(name="xtp", bufs=2) as xtp_pool,
            tc.tile_pool(name="samp", bufs=1) as samp_pool,
            tc.tile_pool(name="eigp", bufs=2) as eig_pool,
            tc.tile_pool(name="work", bufs=3) as work_pool,
            tc.tile_pool(name="small", bufs=2) as small_pool,
            tc.tile_pool(name="osb", bufs=3) as osb_pool,
            tc.tile_pool(name="psops", bufs=3, space="PSUM") as pso_ps,
            tc.tile_pool(name="convps", bufs=1, space="PSUM") as conv_ps,
            tc.tile_pool(name="chainps", bufs=2, space="PSUM") as chain_ps,
        ):
            # ---- load constants into SBUF -------------------------------
            wall_sb = singles.tile([128, 2, 9, 1024], BF16, tag="wall")
            nc.sync.dma_start(out=wall_sb, in_=wall_d.rearrange("c p t o -> p c t o"))
            inver_sb = singles.tile([128, NT, 1, FREQ], F32, tag="inver")
            nc.sync.dma_start(out=inver_sb, in_=inver_d[:])
            mlp_sb = singles.tile([DH, DH], BF16, tag="mlp")
            nc.sync.dma_start(out=mlp_sb, in_=mlp_d[:])
            gamma_sb = singles.tile([DH, FREQ], F32, tag="gamma")
            nc.sync.dma_start(out=gamma_sb, in_=gamma_d[:])
            beta_sb = singles.tile([DH, FREQ], F32, tag="beta")
            nc.sync.dma_start(out=beta_sb, in_=beta_d[:])
            wo_sb = singles.tile([DH, HEADS, C], BF16, tag="wo")
            nc.sync.dma_start(out=wo_sb, in_=wo_d[:])
            brow_sb = singles.tile([1, 1024], BF16, tag="brow")
            nc.sync.dma_start(out=brow_sb, in_=brow_d[:])
            obrow_sb = singles.tile([1, C], BF16, tag="obrow")
            nc.sync.dma_start(out=obrow_sb, in_=obrow_d[:])

            ones_bf = singles.tile([1, 128], BF16, tag="ones_bf")
            nc.gpsimd.memset(ones_bf, 1.0)
            ones_col = singles.tile([64, 1], F32, tag="ones_col")
            nc.gpsimd.memset(ones_col, 1.0)
            ones_row = singles.tile([1, 64], F32, tag="ones_row")
            nc.gpsimd.memset(ones_row, 1.0)
            eps_sb = singles.tile([1, 1], F32, tag="eps")
            nc.gpsimd.memset(eps_sb, EPS)
            ident_f = singles.tile([128, 128], F32, tag="ident_f")
            make_identity(nc, ident_f)
            ident_b = singles.tile([128, 128], BF16, tag="ident_b")
            make_identity(nc, ident_b)

            def stage0(s):
                # load + transpose x[s] into the padded bf16 flat image
                xtp = xtp_pool.tile([128, 2, XLEN], BF16, tag="xtp")
                nc.gpsimd.memset(xtp, 0.0)
                for i, (r0, nr, npos) in enumerate(IN_TILES):
                    n0 = r0 * WW
                    xt = xin_pool.tile([128, C], F32, tag="xt")
                    eng = (nc.sync, nc.gpsimd, nc.scalar)[i % 3]
                    eng.dma_start(out=xt[:npos], in_=x_d[s, n0 : n0 + npos, :])
                    tps0 = pso_ps.tile([128, 2, 128], F32, tag="pso", name="tps0")
                    for q in range(2):
                        nc.tensor.transpose(
                            tps0[:, q, :npos],
                            xt[:npos, q * 128 : (q + 1) * 128],
                            ident_f[:npos, :npos],
                        )
                    base = GUARD + (r0 + 1) * PW
                    dst = xtp[:, :, base : base + nr * PW].rearrange(
                        "p c (r w) -> p c r w", w=PW
                    )[:, :, :, 1 : 1 + WW]
                    nc.vector.tensor_copy(
                        out=dst,
                        in_=tps0[:, :, :npos].rearrange("p c (a b) -> p c a b", b=WW),
                    )
                return xtp

            def transpose_eig(eig_all, etq, t, sz):
                for qq in range(2):
                    tps = pso_ps.tile([128, 2, 128], BF16, tag="pso", name="tps")
                    for q in range(2):
                        nc.tensor.transpose(
                            tps[:, q, :sz],
                            eig_all[:sz, t, (2 * qq + q) * 128 : (2 * qq + q + 1) * 128],
                            ident_b[:sz, :sz],
                        )
                    nc.vector.tensor_copy(
                        out=etq[:, t, 2 * qq : 2 * qq + 2, :sz], in_=tps[:, :, :sz]
                    )

            xtp = stage0(0)
            for s in range(BPC):
                # ---- stage A: conv (fx + logits) + softmax + eig -------
                fx_all = samp_pool.tile([128, NT, INNER], BF16, tag="fx")
                eig_all = eig_pool.tile([128, NT, INNER], BF16, tag="eig")
                for t, (p0, sz) in enumerate(CONV_TILES):
                    cv = [
                        conv_ps.tile([128, 512], F32, tag="cv", name=f"cv{j}")
                        for j in range(2)
                    ]
                    for half in range(2):
                        for ci in range(2):
                            for k, (di, dj) in enumerate(TAPS):
                                o = GUARD + p0 + (di - 1) * PW + (dj - 1)
                                nc.tensor.matmul(
                                    cv[half][:sz, :],
                                    xtp[:, ci, o : o + sz],
                                    wall_sb[:, ci, k, 512 * half : 512 * half + 512],
                                    start=(ci == 0 and k == 0),
                                    stop=not (use_brow or (ci == 1 and k == 8)),
                                )
                        if use_brow:
                            nc.tensor.matmul(
                                cv[half][:sz, :],
                                ones_bf[:, :sz],
                                brow_sb[:, 512 * half : 512 * half + 512],
                                start=False,
                                stop=True,
                            )
                    # fx -> bf16 SBUF (scalar engine, cast on copy)
                    nc.vector.tensor_copy(out=fx_all[:sz, t, :], in_=cv[0][:sz, :])
                    # softmax over g within each head (free-dim segments).
                    # |logits| <= ~3 for this problem, exp is safe unshifted.
                    et = work_pool.tile([128, INNER], F32, tag="et")
                    nc.scalar.activation(
                        out=et[:sz], in_=cv[1][:sz, :],
                        func=mybir.ActivationFunctionType.Exp,
                    )
                    et3 = et[:sz].rearrange("p (h g) -> p h g", h=HEADS)
                    sm = work_pool.tile([128, HEADS], F32, tag="sm")
                    nc.vector.reduce_sum(
                        out=sm[:sz], in_=et3, axis=mybir.AxisListType.X
                    )
                    rc = work_pool.tile([128, HEADS], F32, tag="rc")
                    nc.vector.reciprocal(out=rc[:sz], in_=sm[:sz])
                    nc.vector.tensor_mul(
                        et3, et3, rc[:sz, :, None].to_broadcast((sz, HEADS, FREQ))
                    )
                    # eig = softmax * inver; inver is 0 at garbage centers so
                    # they vanish from both spectral contractions.
                    nc.vector.tensor_mul(
                        eig_all[:sz, t, :].rearrange("p (h g) -> p h g", h=HEADS),
                        et3,
                        inver_sb[:sz, t, :, :].to_broadcast((sz, HEADS, FREQ)),
                    )

                # next sample's input pipeline fills engine gaps from here on
                if s + 1 < BPC:
                    xtp = stage0(s + 1)

                # ---- stages B/C: spec, layernorm, mlp, M_h -------------
                # stage D's eig transposes are interleaved between heads so
                # the PE has filler while each head's LN chain runs on
                # DVE/ACT (chain psum is single-buffered).
                etq = samp_pool.tile([128, NT, 4, 128], BF16, tag="etq")
                mcat = samp_pool.tile([128, 4, C], BF16, tag="mcat")
                tr_sched = [
                    [t for t in range(NT) if t % HEADS == h] for h in range(HEADS)
                ]
                for h in range(HEADS):
                    chain = chain_ps.tile([128, 512], F32, tag="chain")
                    spec_ps = chain[0:DH, 0:FREQ]
                    hs = h * DH
                    for t, (p0, sz) in enumerate(CONV_TILES):
                        nc.tensor.matmul(
                            spec_ps,
                            fx_all[:sz, t, hs : hs + DH],
                            eig_all[:sz, t, hs : hs + FREQ],
                            start=(t == 0),
                            stop=(t == NT - 1),
                        )
                    for t in tr_sched[h]:
                        transpose_eig(eig_all, etq, t, CONV_TILES[t][1])
                    # LayerNorm over all (g, c) jointly; spec_T is (c, g).
                    # ACT computes copy+rowsum and square+rowsum in one op
                    # each via accum_out.
                    spec_sb = small_pool.tile([DH, FREQ], F32, tag="spec_sb")
                    st2 = small_pool.tile([DH, 2], F32, tag="st2")
                    nc.scalar.activation(
                        out=spec_sb, in_=spec_ps,
                        func=mybir.ActivationFunctionType.Copy,
                        accum_out=st2[:, 0:1],
                    )
                    sq = small_pool.tile([DH, FREQ], F32, tag="sq")
                    nc.scalar.activation(
                        out=sq, in_=spec_ps,
                        func=mybir.ActivationFunctionType.Square,
                        accum_out=st2[:, 1:2],
                    )
                    sums_ps = chain[0:1, 64:66]
                    nc.tensor.matmul(sums_ps, ones_col, st2, start=True, stop=True)
                    m2 = small_pool.tile([1, 2], F32, tag="m2")
                    nc.vector.tensor_scalar_mul(m2, sums_ps, 1.0 / (DH * FREQ))
                    mu2 = small_pool.tile([1, 1], F32, tag="mu2")
                    nc.vector.tensor_mul(mu2, m2[:, 0:1], m2[:, 0:1])
                    var = small_pool.tile([1, 1], F32, tag="var")
                    nc.vector.tensor_sub(var, m2[:, 1:2], mu2)
                    sd = small_pool.tile([1, 1], F32, tag="sd")
                    nc.scalar.activation(
                        out=sd, in_=var, func=mybir.ActivationFunctionType.Sqrt,
                        bias=eps_sb, scale=1.0,
                    )
                    ab = small_pool.tile([1, 2], F32, tag="ab")
                    nc.vector.reciprocal(out=ab[:, 0:1], in_=sd)  # rstd
                    t1 = small_pool.tile([1, 1], F32, tag="t1")
                    nc.vector.tensor_mul(t1, m2[:, 0:1], ab[:, 0:1])
                    nc.vector.tensor_scalar_mul(ab[:, 1:2], t1, -1.0)  # -mu*rstd
                    bc_ps = chain[0:DH, 66:68]
                    nc.tensor.matmul(bc_ps, ones_row, ab, start=True, stop=True)
                    ab_col = small_pool.tile([DH, 2], F32, tag="ab_col")
                    nc.vector.tensor_copy(out=ab_col, in_=bc_ps)
                    nc.vector.tensor_scalar(
                        out=spec_sb, in0=spec_sb,
                        scalar1=ab_col[:, 0:1], scalar2=ab_col[:, 1:2],
                        op0=mybir.AluOpType.mult, op1=mybir.AluOpType.add,
                    )
                    nc.vector.tensor_mul(spec_sb, spec_sb, gamma_sb)
                    spec_g = small_pool.tile([DH, FREQ], BF16, tag="spec_g")
                    nc.vector.tensor_add(spec_g, spec_sb, beta_sb)
                    # out_spec_T = mlp_w.T @ spec_T  -> (c_out, g)
                    mlp_ps = chain[0:DH, 128:192]
                    nc.tensor.matmul(mlp_ps, mlp_sb, spec_g, start=True, stop=True)
                    osT = small_pool.tile([DH, FREQ], BF16, tag="osT")
                    nc.scalar.copy(out=osT, in_=mlp_ps)
                    # M_h = out_spec_h @ out_w_h^T  -> (g, 256); heads packed
                    # pairwise into PSUM partitions via column tiling.
                    if h % 2 == 0:
                        mh_ps = pso_ps.tile([128, C], F32, tag="pso", name="mh_ps")
                        nc.tensor.matmul(
                            mh_ps[0:FREQ, :], osT, wo_sb[:, h, :],
                            start=True, stop=True, tile_position=(0, 0),
                        )
                    else:
                        nc.tensor.matmul(
                            mh_ps[FREQ:128, :], osT, wo_sb[:, h, :],
                            start=True, stop=True, tile_position=(0, 64),
                        )
                        nc.scalar.copy(out=mcat[:, h // 2, :], in_=mh_ps)

                # ---- stage D: out = sum_h eig_h @ M_h + out_b ----------
                for t, (p0, sz) in enumerate(CONV_TILES):
                    po = pso_ps.tile([128, C], F32, tag="pso", name="po")
                    for q in range(4):
                        nc.tensor.matmul(
                            po[:sz, :], etq[:, t, q, :sz], mcat[:, q, :],
                            start=(q == 0), stop=not (use_obrow or q == 3),
                        )
                    if use_obrow:
                        nc.tensor.matmul(
                            po[:sz, :], ones_bf[:, :sz], obrow_sb,
                            start=False, stop=True,
                        )
                    ot = osb_pool.tile([128, C], F32, tag="ot")
                    nc.vector.tensor_copy(out=ot[:sz], in_=po[:sz])
                    for i, (off, nrow, ln) in enumerate(OUT_SEGS[t]):
                        eng = (nc.scalar, nc.sync, nc.gpsimd)[(t + i) % 3]
                        eng.dma_start(
                            out=y_d[s, nrow : nrow + ln, :],
                            in_=ot[off : off + ln],
                        )

    nc.compile()
    return nc


def _prep_weights(conv_fx_w, conv_fx_b, conv_x_w, conv_x_b, gate_w, gate_b,
                  temperature, ln_gamma, ln_beta, mlp_w, out_w, out_b, inver):
    import ml_dtypes

    bf = ml_dtypes.bfloat16
    f32 = np.float32
    temp = np.clip(np.asarray(temperature, f32).reshape(HEADS), 0.1, 5.0)
    gw = np.asarray(gate_w, f32)  # (g, d)
    cxw = np.asarray(conv_x_w, f32).reshape(HEADS, DH, C * 9)
    wlg = np.einsum("gd,hdc->hgc", gw, cxw) / temp[:, None, None]
    wlg = wlg.reshape(HEADS, FREQ, C, 3, 3)
    cxb = np.asarray(conv_x_b, f32).reshape(HEADS, DH)
    blg = (np.asarray(gate_b, f32)[None, :] + np.einsum("gd,hd->hg", gw, cxb))
    blg = blg / temp[:, None]

    wfull = np.concatenate(
        [np.asarray(conv_fx_w, f32), wlg.reshape(INNER, C, 3, 3)], axis=0
    )  # (1024, 256, 3, 3)
    wfull = wfull.reshape(1024, 2, 128, 9)
    wall = np.ascontiguousarray(np.transpose(wfull, (1, 2, 3, 0))).astype(bf)

    bias_row = np.concatenate(
        [np.asarray(conv_fx_b, f32), blg.reshape(INNER)]
    ).reshape(1, 1024).astype(bf)

    # inver scattered onto the padded flat grid, zero at garbage centers.
    inver_np = np.asarray(inver, f32)
    inver_flat = np.zeros((NP, FREQ), f32)
    grid = inver_flat.reshape(PH, PW, FREQ)
    grid[1 : 1 + HH, 1 : 1 + WW, :] = inver_np.reshape(HH, WW, FREQ)
    inver_p = np.zeros((128, NT, 1, FREQ), f32)
    for t, (p0, sz) in enumerate(CONV_TILES):
        inver_p[:sz, t, 0, :] = inver_flat[p0 : p0 + sz]

    wo_t = np.ascontiguousarray(
        np.asarray(out_w, f32).reshape(C, HEADS, DH).transpose(2, 1, 0)
    ).astype(bf)

    return dict(
        wall=wall,
        inver_p=inver_p,
        mlp_b=np.asarray(mlp_w, f32).astype(bf),
        gamma_t=np.ascontiguousarray(np.asarray(ln_gamma, f32).T),
        beta_t=np.ascontiguousarray(np.asarray(ln_beta, f32).T),
        wo_t=wo_t,
        bias_row=bias_row,
        outb_row=np.asarray(out_b, f32).reshape(1, C).astype(bf),
    )


_NC_CACHE = {}


def get_nc(use_brow=True, use_obrow=True):
    key = (use_brow, use_obrow)
    if key not in _NC_CACHE:
        _NC_CACHE[key] = _build_nc(*key)
    return _NC_CACHE[key]


def run(inputs, trace=False, tmpdir=None):
    """Run on 8 cores; returns (output (32,N,C) f32, BassKernelResults)."""
    x = np.asarray(inputs["x"], np.float32)
    assert x.shape == (B, N, C)
    consts = _prep_weights(**{k: v for k, v in inputs.items() if k != "x"})
    in_maps = [
        {"x": np.ascontiguousarray(x[i * BPC : (i + 1) * BPC]), **consts}
        for i in range(NCORES)
    ]
    nc = get_nc(
        use_brow=bool(np.any(consts["bias_row"])),
        use_obrow=bool(np.any(consts["outb_row"])),
    )
    res = run_bass_kernel_spmd(
        nc, in_maps, list(range(NCORES)), trace=trace, tmpdir=tmpdir
    )
    out = np.concatenate([res.results[i]["y"] for i in range(NCORES)], axis=0)
    return out.astype(np.float32), res


def kernel(**inputs) -> np.ndarray:
    out, _ = run(inputs, trace=False)
    return out
